# revision 2
# baseline (speedup 1.0000x reference)
"""GQA causal attention (ternary weights) on 8 TRN2 NeuronCores — v2.

v2 changes vs baseline:
  - q/k/v projections: fp8 e4m3 3-level x (host-split, scales 1/16/256 folded
    into exact ternary weight level copies) with DoubleRow matmuls (2 chunks
    of contraction per pass at 0.5 cyc/col).
  - PV: p stored fp8 (exp output), v stored as 2 unscaled fp8 levels;
    DoubleRow over k-chunk pairs.
  - o_proj: attention output stored as 2 unscaled fp8 levels; DoubleRow over
    the 2 contraction chunks.
  - S^T exact pass and the fp16 max-estimate pass unchanged (precision).
  - engine rebalance: qA/qB/klkh fp16 prep on Pool, fused scale-sub on DVE.
"""

import sys

sys.path.insert(0, "/opt/trn_rl_repo")

import numpy as np
import ml_dtypes

B = 2
S = 2048
D = 2048
NCORES = 8
HEADS_PER_CORE = 4
HD = 64
QROWS = HEADS_PER_CORE * HD  # 256
TT = 512  # token tile
LO_SCALE = 1024.0
MASK_NEG = -30000.0
E4NP = ml_dtypes.float8_e4m3

_CACHE = {}


def _build_program(b=B, s=S, d=D):
    import concourse.bacc as bacc
    import concourse.tile as tile
    import concourse.mybir as mybir
    from concourse import masks
    from contextlib import ExitStack

    f32 = mybir.dt.float32
    f32r = mybir.dt.float32r
    f16 = mybir.dt.float16
    f8 = mybir.dt.float8e4
    f8w = mybir.dt.float8e5   # wide-range fp8 for softmax probabilities
    Alu = mybir.AluOpType
    Act = mybir.ActivationFunctionType
    DR = mybir.MatmulPerfMode.DoubleRow

    tokens = b * s
    n_tt = tokens // TT          # token tiles
    tt_per_b = s // TT
    n_dc = d // 128              # contraction chunks for projections
    n_qt = s // TT               # 512-wide q tiles per batch
    n_qc = s // 128              # 128-wide q chunks per batch (max pass)
    n_mt = d // 128              # output row tiles for o_proj
    n_oc = QROWS // 128          # o_proj contraction chunks (2)
    sub = TT // 128              # 128-sub-blocks per 512 tile (4)
    n_ch = tokens // 128         # 128-token k chunks over both batches

    nc = bacc.Bacc("TRN2", target_bir_lowering=False, debug=False,
                   num_devices=NCORES)

    x_d = [nc.dram_tensor(f"x{l}", [d, tokens], f8, kind="ExternalInput").ap()
           for l in range(3)]
    wq_d = [nc.dram_tensor(f"wq{l}", [d, QROWS], f8,
                           kind="ExternalInput").ap() for l in range(3)]
    wkv_d = [nc.dram_tensor(f"wkv{l}", [d, 128], f8,
                            kind="ExternalInput").ap() for l in range(3)]
    wo_d = nc.dram_tensor("wo", [QROWS, d], f8, kind="ExternalInput").ap()
    out_d = nc.dram_tensor("out", [d, tokens], f32, kind="ExternalOutput").ap()

    with tile.TileContext(nc) as tc, ExitStack() as top:
        constp = top.enter_context(tc.tile_pool(name="const", bufs=1))
        wpool = top.enter_context(tc.tile_pool(name="wts", bufs=1))
        pp = top.enter_context(tc.tile_pool(name="persist", bufs=1))

        # --- constants -------------------------------------------------
        maskM = constp.tile([128, 128], f32, tag="maskM")   # [k,q]: keep k<=q
        nc.gpsimd.memset(maskM[:], 0.0)
        nc.gpsimd.affine_select(
            out=maskM[:], in_=maskM[:], compare_op=Alu.is_ge, fill=MASK_NEG,
            base=0, pattern=[[1, 128]], channel_multiplier=-1)
        maskM2 = constp.tile([128, 128], f32, tag="maskM2")  # [q,k]: keep k<=q
        nc.gpsimd.memset(maskM2[:], 0.0)
        nc.gpsimd.affine_select(
            out=maskM2[:], in_=maskM2[:], compare_op=Alu.is_ge, fill=MASK_NEG,
            base=0, pattern=[[-1, 128]], channel_multiplier=1)
        ident = constp.tile([128, 128], f32, tag="ident")
        masks.make_identity(nc, ident[:])
        onesc = constp.tile([65, HD], f32r, tag="onesc")
        nc.scalar.activation(onesc[:], maskM[0:65, 0:HD], Act.Identity,
                             bias=1.0, scale=0.0)

        # --- weights ---------------------------------------------------
        wq_sb = []
        for l in range(3):
            t = wpool.tile([128, n_dc * QROWS], f8, tag=f"wq{l}",
                           name=f"wq{l}")
            nc.sync.dma_start(
                out=t[:].rearrange("p (c n) -> p c n", n=QROWS),
                in_=wq_d[l].rearrange("(c p) n -> p c n", p=128))
            wq_sb.append(t)
        wkv_sb = []
        for l in range(3):
            t = wpool.tile([128, n_dc * 128], f8, tag=f"wkv{l}",
                           name=f"wkv{l}")
            nc.sync.dma_start(
                out=t[:].rearrange("p (c n) -> p c n", n=128),
                in_=wkv_d[l].rearrange("(c p) n -> p c n", p=128))
            wkv_sb.append(t)

        zcol = constp.tile([128, 1], f32, tag="zcol")
        nc.gpsimd.memset(zcol[:], 0.0)
        # maskE: zeros except last 128 cols = maskM2 (diag [q,k] mask)
        maskE = constp.tile([128, TT], f32, tag="maskE")
        nc.gpsimd.memset(maskE[:], 0.0)
        nc.gpsimd.tensor_copy(maskE[:, TT - 128:TT], maskM2[:])

        # --- persistent activations -----------------------------------
        qA = [pp.tile([65, tokens], f16, tag=f"qA{h}", name=f"qA{h}")
              for h in range(HEADS_PER_CORE)]
        # qB (fp8): rows 0:64 = qh/32, rows 64:128 = 32*(q/8 - qh)
        qB = [pp.tile([128, tokens], f8, tag=f"qB{h}", name=f"qB{h}")
              for h in range(HEADS_PER_CORE)]
        khb = pp.tile([65, tokens], f16, tag="khb")
        # klkh (fp8, zero-padded DoubleRow lhsT): half j=0 rows 0:64 =
        # 32*(k-kh), rows 64:128 = kh/32; half j=1 all zeros.
        klkh = pp.tile([128, 2 * tokens], f8, tag="klkh")
        nc.gpsimd.memset(klkh[:, tokens:2 * tokens], 0.0)
        # vhat levels: [128, (chunk 65)]; col 64 = 1 (lvl1) / 0 (lvl2)
        vhat1 = pp.tile([128, n_ch * 65], f8, tag="vhat1")
        vhat2 = pp.tile([128, n_ch * 65], f8, tag="vhat2")
        nc.gpsimd.memset(vhat2[:], 0.0)
        nc.scalar.activation(
            vhat1[:], maskM[:, 0:1].to_broadcast([128, n_ch * 65]),
            Act.Identity, bias=1.0, scale=0.0)
        nc.gpsimd.memset(khb[64:65, :], -1.0)

        def vh1(pc):
            return vhat1[:].rearrange("p (c j n) -> p c j n", j=2, n=65)[:, pc]

        def vh2(pc):
            return vhat2[:].rearrange("p (c j n) -> p c j n", j=2, n=65)[:, pc]

        with ExitStack() as ph:
            mp = ph.enter_context(tc.tile_pool(name="mp", bufs=2))
            ps1 = ph.enter_context(
                tc.tile_pool(name="ps1", bufs=3, space="PSUM"))
            psst = ph.enter_context(
                tc.tile_pool(name="psst", bufs=2, space="PSUM"))
            psav = ph.enter_context(
                tc.tile_pool(name="psav", bufs=2, space="PSUM"))
            psbc = ph.enter_context(
                tc.tile_pool(name="psbc", bufs=1, space="PSUM"))

            # ---------- S~ max-estimate pass (fp16, unchanged) -----------
            mstate = {}
            mbp = ph.enter_context(tc.tile_pool(name="mbp", bufs=8))

            def s_block(bb, h, qc):
                # Row-max estimate for q-chunk qc: fused (mask-)add +
                # max-reduce on DVE with chained initial value (no second
                # reduce stage; one op per 512-col block).
                boff = bb * s
                if qc == 0:
                    mstate[(bb, h)] = mbp.tile([128, n_qc], f32, tag="mbuf",
                                               name="mbuf")
                mbuf = mstate[(bb, h)]
                qsl = slice(boff + qc * 128, boff + qc * 128 + 128)
                ntk = qc // sub + 1
                for kt in range(ntk):
                    w = min(TT, (qc + 1) * 128 - kt * TT)
                    st = psst.tile([128, TT], f32, tag="st")
                    nc.tensor.matmul(
                        st[:, 0:w],
                        lhsT=qA[h][0:64, qsl],
                        rhs=khb[0:64, boff + kt * TT:boff + kt * TT + w],
                        start=True, stop=True)
                    diag = kt == ntk - 1
                    in1 = (maskE[:, TT - w:TT] if diag else
                           zcol[:].to_broadcast([128, w]))
                    nc.vector.tensor_tensor_reduce(
                        out=st[:, 0:w], in0=st[:, 0:w], in1=in1,
                        scale=1.0,
                        scalar=(-30000.0 if kt == 0
                                else mbuf[:, qc:qc + 1]),
                        op0=Alu.add, op1=Alu.max,
                        accum_out=mbuf[:, qc:qc + 1])

            def s_final(bb, h):
                boff = bb * s
                mbuf = mstate.pop((bb, h))
                mps = psst.tile([128, TT], f32, tag="st")
                nc.tensor.transpose(mps[0:n_qc, 0:128], mbuf[:, 0:n_qc],
                                    ident[:, 0:128])
                mrow = mp.tile([n_qc, 128], f32, tag="mrow")
                # +2 margin: caps exp(s-m~) (fp8 range) both ways; the
                # softmax normalization cancels the constant.
                nc.vector.tensor_scalar_add(mrow[:], mps[0:n_qc, 0:128], 2.0)
                nc.gpsimd.dma_start(
                    out=qA[h][64:65, boff:boff + s].rearrange(
                        "o (c t) -> o c t", t=128),
                    in_=mrow[:])

            # ================= phase 1: projections ====================
            ph1 = ExitStack()
            xp = ph1.enter_context(tc.tile_pool(name="xp", bufs=2))
            sp1 = ph1.enter_context(tc.tile_pool(name="sp1", bufs=3))
            for tt in range(n_tt):
                tcols = slice(tt * TT, (tt + 1) * TT)
                x_sb = []
                for l in range(3):
                    t = xp.tile([128, n_dc * TT], f8, tag=f"x{l}",
                                name="xtile")
                    nc.sync.dma_start(
                        out=t[:].rearrange("p (c t) -> p c t", t=TT),
                        in_=x_d[l].rearrange("(c p) t -> p c t",
                                             p=128)[:, :, tcols])
                    x_sb.append(t)

                def proj(w_sb, mcol, mwid, ps):
                    n_pr = n_dc // 2
                    for l in range(3):
                        wv = w_sb[l][:].rearrange("p (c n) -> p c n", n=mwid)
                        xv = x_sb[l][:].rearrange("p (c t) -> p c t", t=TT)
                        for pr in range(n_pr):
                            for hf in range(2):
                                nc.tensor.matmul(
                                    ps[:, hf * 256:(hf + 1) * 256],
                                    lhsT=wv[:, 2 * pr:2 * pr + 2,
                                            mcol:mcol + 128],
                                    rhs=xv[:, 2 * pr:2 * pr + 2,
                                           hf * 256:(hf + 1) * 256],
                                    start=(l == 0 and pr == 0),
                                    stop=(l == 2 and pr == n_pr - 1),
                                    perf_mode=DR, skip_group_check=True)

                for m in range(QROWS // 128):
                    ps = ps1.tile([128, TT], f32, tag="ps")
                    proj(wq_sb, m * 128, QROWS, ps)
                    qh16 = sp1.tile([128, TT], f16, tag="qh16")
                    nc.scalar.activation(qh16[:], ps[:], Act.Copy,
                                         scale=0.125)
                    res = sp1.tile([128, TT], f32, tag="qres")
                    nc.vector.scalar_tensor_tensor(
                        out=res[:], in0=ps[:], scalar=0.125, in1=qh16[:],
                        op0=Alu.mult, op1=Alu.subtract)
                    for i in range(2):
                        h = 2 * m + i
                        rows = slice(i * 64, i * 64 + 64)
                        nc.scalar.copy(qA[h][0:64, tcols], qh16[rows, :])
                        nc.scalar.mul(qB[h][0:64, tcols], qh16[rows, :],
                                      1.0 / 32.0)
                        nc.scalar.mul(qB[h][64:128, tcols], res[rows, :],
                                      32.0)

                ps = ps1.tile([128, TT], f32, tag="ps")
                proj(wkv_sb, 0, 128, ps)
                nc.scalar.copy(khb[0:64, tcols], ps[0:64, :])
                nc.gpsimd.tensor_scalar_mul(
                    klkh[64:128, tcols], khb[0:64, tcols], 1.0 / 32.0)
                res = sp1.tile([128, TT], f32, tag="qres")
                nc.vector.tensor_tensor(
                    res[0:64, :], ps[0:64, :], khb[0:64, tcols],
                    op=Alu.subtract)
                nc.gpsimd.tensor_scalar_mul(
                    klkh[0:64, tcols], res[0:64, :], 32.0)
                vtmp = sp1.tile([64, TT], f32, tag="vtmp")
                nc.scalar.mul(vtmp[:], ps[64:128, :], 0.5)  # v/2: e4m3 range
                for j in range(sub):
                    ptr = psst.tile([128, TT], f32, tag="st")
                    nc.tensor.transpose(ptr[0:128, 0:64],
                                        vtmp[:, j * 128:(j + 1) * 128],
                                        ident[0:64, 0:64])
                    ch = tt * sub + j
                    co = ch * 65
                    nc.scalar.copy(vhat1[:, co:co + 64], ptr[0:128, 0:64])
                    nc.vector.tensor_tensor(
                        vhat2[:, co:co + 64], ptr[0:128, 0:64],
                        vhat1[:, co:co + 64], op=Alu.subtract)
                # batch-0 S~ blocks for the q-chunks this tile enabled
                bb, ltt = tt // tt_per_b, tt % tt_per_b
                if bb == 0:
                    for h in range(HEADS_PER_CORE):
                        for qc in range(ltt * sub, (ltt + 1) * sub):
                            s_block(bb, h, qc)
                    if ltt == tt_per_b - 1:
                        for h in range(HEADS_PER_CORE):
                            s_final(bb, h)

            ph1.close()
            # ============ phase 2 + per-batch o_proj ====================
            aop = ph.enter_context(tc.tile_pool(name="aop", bufs=1))
            ptp = ph.enter_context(tc.tile_pool(name="ptp", bufs=3))
            outp = ph.enter_context(tc.tile_pool(name="outp", bufs=6))
            # ao levels: [128, (j tok)]: j=0 -> heads 0,1 dims; j=1 -> 2,3
            ao1 = aop.tile([128, 2 * tokens], f8, tag="ao1", name="ao1")
            ao2 = aop.tile([128, 2 * tokens], f8, tag="ao2", name="ao2")
            wo_sb = aop.tile([128, n_oc * d], f8, tag="wo")
            nc.sync.dma_start(
                out=wo_sb[:].rearrange("p (c n) -> p c n", n=d),
                in_=wo_d.rearrange("(c p) n -> p c n", p=128))
            wov = wo_sb[:].rearrange("p (c n) -> p c n", n=d)

            def aov(t, j):
                return t[:].rearrange("p (j t) -> p j t", j=2)[:, j]

            def av(pav, pt, lo, bb, pc, npairs):
                # DoubleRow PV over a chunk pair; both levels; <=256-col segs
                pch = bb * (s // 256) + pc
                segs = []
                x0 = lo
                while x0 < TT:
                    x1 = min(x0 + 256, TT)
                    segs.append((x0, x1))
                    x0 = x1
                for li, vt in ((0, vh1(pch)), (1, vh2(pch))):
                    for si, (a0, a1_) in enumerate(segs):
                        nc.tensor.matmul(
                            pav[:, a0:a1_], lhsT=vt,
                            rhs=pt[:, :, a0:a1_],
                            start=(pc == 0 and li == 0 and si == 0),
                            stop=(pc == npairs - 1 and li == 1
                                  and si == len(segs) - 1),
                            perf_mode=DR, skip_group_check=True)

            def oproj_part(bb, m, qt0, nqt):
                boff = bb * s
                osb = outp.tile([128, nqt * TT], f32, tag="ot", name="osb")
                for i in range(nqt):
                    po = ps1.tile([128, TT], f32, tag="ps")
                    for l, aot in ((0, ao1), (1, ao2)):
                        for hf in range(2):
                            c0 = boff + (qt0 + i) * TT + hf * 256
                            nc.tensor.matmul(
                                po[:, hf * 256:(hf + 1) * 256],
                                lhsT=wov[:, 0:2, m * 128:m * 128 + 128],
                                rhs=aot[:].rearrange("p (j t) -> p j t",
                                                     j=2)[:, :, c0:c0 + 256],
                                start=(l == 0), stop=(l == 1),
                                perf_mode=DR, skip_group_check=True)
                    nc.vector.tensor_copy(osb[:, i * TT:(i + 1) * TT], po[:])
                nc.sync.dma_start(
                    out=out_d[m * 128:(m + 1) * 128,
                              boff + qt0 * TT:boff + (qt0 + nqt) * TT],
                    in_=osb[:])

            b1q = []
            for h in range(HEADS_PER_CORE):
                for qc in range(n_qc):
                    b1q.append(lambda h=h, qc=qc: s_block(1, h, qc))
                b1q.append(lambda h=h: s_final(1, h))
            wsum = HEADS_PER_CORE * n_qt * (n_qt + 1) // 2
            bcum, acc = [], 0.0
            for h in range(HEADS_PER_CORE):
                for qt in range(n_qt):
                    acc += (qt + 1) * len(b1q) / wsum
                    bcum.append(min(int(round(acc)), len(b1q)))
            bcum[-1] = len(b1q)

            slot = 0
            for bb in range(b):
                boff = bb * s
                for h in range(HEADS_PER_CORE):
                    for qt in range(n_qt):
                        qlo = boff + qt * TT
                        pav = psav.tile([65, TT], f32, tag="pav")
                        nchunks = (qt + 1) * sub
                        npairs = nchunks // 2
                        prev = None
                        for pc in range(npairs):
                            pt = ptp.tile([128, 2 * TT], f8w, tag="pt")
                            ptv = pt[:].rearrange("p (j t) -> p j t", j=2)
                            lo_p = None
                            for j in range(2):
                                kc = 2 * pc + j
                                ksl = slice(boff + kc * 128,
                                            boff + kc * 128 + 128)
                                jj = kc - qt * sub
                                lo = max(jj, 0) * 128
                                if j == 0:
                                    lo_p = lo
                                w = TT - lo
                                s2 = ps1.tile([128, TT], f32, tag="ps")
                                nc.tensor.matmul(
                                    s2[:, lo:lo + w], lhsT=khb[:, ksl],
                                    rhs=qA[h][:, qlo + lo:qlo + TT],
                                    start=True, stop=False,
                                    skip_group_check=True)
                                klv = klkh[:].rearrange(
                                    "p (j t) -> p j t", j=2)[:, :, ksl]
                                x0 = lo
                                while x0 < TT:
                                    x1_ = min(x0 + 256, TT)
                                    nc.tensor.matmul(
                                        s2[:, x0:x1_], lhsT=klv,
                                        rhs=qB[h][:, qlo + x0:qlo + x1_]
                                        .rearrange("p (j t) -> p j t", j=1)
                                        .to_broadcast([128, 2, x1_ - x0]),
                                        start=False, stop=(x1_ == TT),
                                        perf_mode=DR, skip_group_check=True)
                                    x0 = x1_
                                nc.scalar.activation(
                                    ptv[:, j, lo:lo + w],
                                    s2[:, lo:lo + w], Act.Exp)
                                if jj >= 0:
                                    # zero masked (k>q) entries of the diag
                                    # block post-exp (GPSIMD is SBUF-only)
                                    nc.gpsimd.affine_select(
                                        out=ptv[:, j, lo:lo + 128],
                                        in_=ptv[:, j, lo:lo + 128],
                                        compare_op=Alu.is_ge, fill=0.0,
                                        base=0, pattern=[[1, 128]],
                                        channel_multiplier=-1)
                                if j == 1 and lo > lo_p:
                                    nc.gpsimd.memset(
                                        ptv[:, 1, lo_p:lo], 0.0)
                            if prev is not None:
                                av(pav, prev[0], prev[1], bb, prev[2],
                                   npairs)
                            prev = (ptv, lo_p, pc)
                        av(pav, prev[0], prev[1], bb, prev[2], npairs)

                        rec = mp.tile([65, TT], f32r, tag="rec")
                        with nc.allow_low_precision(
                                reason="1/l broadcast feeds matmul"):
                            nc.vector.reciprocal(rec[64:65, :],
                                                 pav[64:65, :])
                        bc = psbc.tile([64, TT], f32, tag="bc")
                        nc.tensor.matmul(
                            bc[:], lhsT=onesc[64:65, 0:HD],
                            rhs=rec[64:65, :], start=True, stop=True)
                        aof = mp.tile([64, TT], f32, tag="aof")
                        nc.vector.tensor_tensor(
                            aof[:], pav[0:64, :], bc[:], op=Alu.mult)
                        rows = slice((h % 2) * 64, (h % 2) * 64 + 64)
                        a1v = aov(ao1, h // 2)[rows, qlo:qlo + TT]
                        nc.scalar.copy(a1v, aof[:])
                        nc.gpsimd.tensor_tensor(
                            aov(ao2, h // 2)[rows, qlo:qlo + TT],
                            aof[:], a1v, op=Alu.subtract)

                        if bb == 0:
                            lo_i = bcum[slot - 1] if slot else 0
                            for fn in b1q[lo_i:bcum[slot]]:
                                fn()
                            slot += 1
                        else:
                            if h < HEADS_PER_CORE - 1:
                                # b0 o_proj spread over h<3 slots
                                si = (h * n_qt + qt)
                                nsl = (HEADS_PER_CORE - 1) * n_qt
                                per = (n_mt + nsl - 1) // nsl
                                for m in range(si * per,
                                               min((si + 1) * per, n_mt)):
                                    oproj_part(0, m, 0, n_qt)
                            else:
                                # b1 o_proj per finished qt strip (all
                                # heads of this qt done once h==3 passes)
                                for m in range(n_mt):
                                    oproj_part(1, m, qt, 1)

    nc.compile()
    return nc


def _ternarize(w):
    w = np.asarray(w, np.float32)
    scale = max(np.abs(w).mean(), 1e-6)
    return ((w > 0.05 * scale).astype(np.float32)
            - (w < -0.05 * scale).astype(np.float32))


def _split_x_fp8(xT):
    x1 = xT.astype(E4NP)
    r = xT - x1.astype(np.float32)
    x2 = (r * 16.0).astype(E4NP)
    r = r - x2.astype(np.float32) / 16.0
    x3 = (r * 256.0).astype(E4NP)
    return x1, x2, x3


def kernel(x, wq, wk, wv, wo):
    from concourse.bass_utils import run_bass_kernel_spmd

    if "nc" not in _CACHE:
        _CACHE["nc"] = _build_program()
    nc = _CACHE["nc"]

    tq = _ternarize(wq)
    tk = _ternarize(wk)
    tv = _ternarize(wv)
    to = _ternarize(wo)

    xT = np.ascontiguousarray(np.asarray(x, np.float32).reshape(B * S, D).T)
    x1, x2, x3 = _split_x_fp8(xT)

    in_maps = []
    for c in range(NCORES):
        qsl = slice(c * QROWS, (c + 1) * QROWS)
        ksl = slice(c * HD, (c + 1) * HD)
        wkv = np.concatenate([tk[ksl], tv[ksl]], axis=0)  # [128, D]
        wqT = np.ascontiguousarray(tq[qsl].T)
        wkvT = np.ascontiguousarray(wkv.T)
        # ao levels hold out/2 (v stored halved for e4m3 range) -> double wo
        im = {"x0": x1, "x1": x2, "x2": x3,
              "wo": np.ascontiguousarray(2.0 * to[:, qsl].T).astype(E4NP)}
        for l, sc in enumerate((1.0, 1.0 / 16.0, 1.0 / 256.0)):
            im[f"wq{l}"] = (wqT * sc).astype(E4NP)
            im[f"wkv{l}"] = (wkvT * sc).astype(E4NP)
        in_maps.append(im)

    res = run_bass_kernel_spmd(nc, in_maps, list(range(NCORES)))
    total = res.results[0]["out"]
    for c in range(1, NCORES):
        total = total + res.results[c]["out"]
    return np.ascontiguousarray(total.T).reshape(B, S, D).astype(np.float32)
